# revision 1
# baseline (speedup 1.0000x reference)
"""Multi-head attention (B=4, L=2048, E=1024, H=16, DK=64) on 8 TRN2 cores.

Sharding: core c -> (batch b = c//2, head-group g = c%2 of 8 heads).

Single fused instruction stream per core: one software pipeline over 256
attention steps (4 query-quarters x 4 head-pairs x 16 key-tiles). Every
matmul in the kernel is a PE tile-shape (64,128) op, and consecutive
matmuls are paired on complementary partition halves (positions (0,0) /
(64,0)) with identical moving columns so the PE fuses each pair into a
single 512-cycle pass. Per step: fused ST pass (2 heads) -> exp[128,1024]
(ACT) -> 2 fused AV passes (key-halves accumulate into one PSUM bank via
two fixed-position groups). QKV/FC chains drip into the stream as
background PE work; per-quarter partial FC outputs are pairwise
ReduceScattered (bf16) while later quarters compute. Host casts to f32.

Self-contained: hardcodes all shapes; requires only the concourse stack.
"""

import numpy as np
import ml_dtypes

try:
    import axon_prof

    axon_prof.install()
except Exception:
    pass

import concourse.mybir as mybir
import concourse.tile as tile
from concourse import bacc
from concourse import bass_utils

B, L, E = 4, 2048, 1024
H, DK = 16, 64
H8 = 8                      # heads per core
F = H8 * 3 * DK             # qkv features per core = 1536
FO = H8 * DK                # attn-out features per core = 512
NCORES = 8
Q4 = L // 4                 # 512 queries per quarter
Q8 = Q4 // 2                # 256 tokens scattered to each pair member

# ft-tile order: Q tiles 0..3 (head-pairs), K tiles 4..7, V tiles 8..11.
# Tiles {0,4,8} (head-pair 0) ship in w_pre so block (0,0) starts early.
PRE_FTS = (0, 4, 8)
REST_FTS = (1, 2, 3, 5, 6, 7, 9, 10, 11)
PRE_IDX = {ft: i for i, ft in enumerate(PRE_FTS)}
REST_IDX = {ft: i for i, ft in enumerate(REST_FTS)}

f32 = mybir.dt.float32
bf16 = mybir.dt.bfloat16
Exp = mybir.ActivationFunctionType.Exp
MUL = mybir.AluOpType.mult
ADD = mybir.AluOpType.add

_CACHE = {}


def build_nc():
    nc = bacc.Bacc("TRN2", target_bir_lowering=False, debug=False, num_devices=NCORES)

    # weight tensors arrive host-prearranged in SBUF layout (partition-major)
    # so every input DMA is contiguous per partition at full HBM rate
    x = nc.dram_tensor("x", [E, L], bf16, kind="ExternalInput")
    w_pre = nc.dram_tensor("w_pre", [128, 8 * 384], bf16, kind="ExternalInput")
    w_rest = nc.dram_tensor("w_rest", [128, 8 * 1152], bf16, kind="ExternalInput")
    b_qkv = nc.dram_tensor("b_qkv", [128, 12], f32, kind="ExternalInput")
    w_fc = nc.dram_tensor("w_fc", [128, 4 * E], bf16, kind="ExternalInput")
    b_fc = nc.dram_tensor("b_fc", [1, E], f32, kind="ExternalInput")
    # 4 quarters x 256 scattered tokens each; host casts bf16 -> f32
    out = nc.dram_tensor("out", [L // 2, E], bf16, kind="ExternalOutput")

    PAIRS = [[0, 1], [2, 3], [4, 5], [6, 7]]

    with tile.TileContext(nc) as tc:
        with (
            tc.tile_pool(name="persist", bufs=1) as pp,
            tc.tile_pool(name="work", bufs=2) as wp,
            tc.tile_pool(name="ys", bufs=3) as yp_pool,
            tc.tile_pool(name="stp", bufs=2, space="PSUM") as stp,
            tc.tile_pool(name="avp", bufs=1, space="PSUM") as avp,
            tc.tile_pool(name="qp", bufs=2, space="PSUM") as qp,
            tc.tile_pool(name="dram", bufs=1, space="DRAM") as dram,
        ):
            # ---- persistent SBUF ----
            xT = pp.tile([128, 8, L], bf16, tag="xT")          # X^T  4 MiB
            wqp = pp.tile([128, 8, 384], bf16, tag="wqp")
            wqr = pp.tile([128, 8, 1152], bf16, tag="wqr")
            bq = pp.tile([128, 12], f32, tag="bq")
            wfc = pp.tile([128, 4, E], bf16, tag="wfc")        # 1 MiB
            bias = pp.tile([128, E], f32, tag="bias")          # 0.5 MiB
            qt = pp.tile([128, 4, L], bf16, tag="qt")          # Q^T 2 MiB
            kt = pp.tile([128, 4, L], bf16, tag="kt")          # K^T 2 MiB
            vt = pp.tile([128, 4, L], bf16, tag="vt")          # V^T 2 MiB
            # V natural layout, 80-elem stride; col 64 holds the ones column
            # so AV matmuls with lhsT [V|1] (M=65) produce rowsums for free
            v = pp.tile([128, H8, 16, 80], bf16, tag="v")      # 2.5 MiB
            onT = pp.tile([128, 4, L], bf16, tag="onT")        # attn out^T 2 MiB

            rs_in = [
                dram.tile([Q4, E], bf16, name=f"rs_in{i}", tag=f"rs_in{i}")
                for i in range(4)
            ]
            rs_out = [
                dram.tile([Q8, E], bf16, name=f"rs_out{i}", tag=f"rs_out{i}")
                for i in range(4)
            ]

            # ---- input DMAs, all on the sync (SP) queue: the ACT queue must
            # stay empty or exp issue stalls behind DMA triggers. Small
            # tensors go first so nothing waits behind a bulk transfer;
            # wfc is deferred into the background schedule. ----
            nc.scalar.dma_start(bq[:], b_qkv[:])
            bfc_row = pp.tile([1, E], f32, tag="bfc_row")
            nc.scalar.dma_start(bfc_row[:], b_fc[:])
            nc.sync.dma_start(xT[:, 0, :], x[0:128, :])
            for i in range(3):
                nc.sync.dma_start(
                    wqp[:, :, i * 128 : (i + 1) * 128],
                    w_pre[:, :].rearrange("p (a f) -> p a f", a=8)[
                        :, :, i * 128 : (i + 1) * 128
                    ],
                )
            for e in (2, 4, 6):
                nc.sync.dma_start(xT[:, e, :], x[e * 128 : (e + 1) * 128, :])
            for e in (1, 3, 5, 7):
                nc.scalar.dma_start(xT[:, e, :], x[e * 128 : (e + 1) * 128, :])
            nc.sync.dma_start(wqr[:], w_rest[:])
            nc.gpsimd.partition_broadcast(bias[:], bfc_row[:])
            nc.vector.memset(v[:, :, :, 64:65], 1.0)

            def wq_sl(ft):
                if ft in PRE_IDX:
                    i = PRE_IDX[ft]
                    return lambda kc: wqp[:, kc, i * 128 : (i + 1) * 128]
                i = REST_IDX[ft]
                return lambda kc: wqr[:, kc, i * 128 : (i + 1) * 128]

            # ---- emitters: every matmul is a (64,128)-shape op; pairs at
            # positions (0,0)/(64,0) with identical moving columns fuse ----
            def qkv_chain(ft, tb):
                ps = qp.tile([128, 512], f32, tag="ps", name="ps")
                wa = wq_sl(ft)
                tsl = slice(tb * 512, (tb + 1) * 512)
                for kc in range(8):
                    nc.tensor.matmul(
                        ps[:], wa(kc), xT[:, kc, tsl],
                        start=(kc == 0), stop=(kc == 7),
                    )
                if ft < 4:
                    dst = qt[:, ft, tsl]
                elif ft < 8:
                    dst = kt[:, ft - 4, tsl]
                else:
                    dst = vt[:, ft - 8, tsl]
                nc.vector.tensor_scalar_add(dst, ps[:], bq[:, ft : ft + 1])

            def v_transpose(p, tb):
                # V^T -> V (token-major) via xbar transpose, per head, per
                # 512-token slice (4 key-chunks)
                for h in (2 * p, 2 * p + 1):
                    nc.sync.dma_start_transpose(
                        v[:, h, tb * 4 : (tb + 1) * 4, 0:DK],
                        vt[(h % 2) * 64 : (h % 2) * 64 + 64, p, tb * 512 : (tb + 1) * 512],
                    )

            def fc_chain(qq, tc_i, e2):
                # tokens (qq*4 + tc_i)*128 .. +128, output cols e2*512 .. +512
                t0 = (qq * 4 + tc_i) * 128
                esl = slice(e2 * 512, (e2 + 1) * 512)
                yp = qp.tile([128, 512], f32, tag="ps", name="yp")
                for c in range(4):
                    nc.tensor.matmul(
                        yp[:], onT[:, c, t0 : t0 + 128], wfc[:, c, esl],
                        start=(c == 0), stop=(c == 3),
                    )
                ys = yp_pool.tile([128, 512], bf16, tag="ys", name="ys")
                nc.vector.tensor_tensor(ys[:], yp[:], bias[:, esl], op=ADD)
                nc.sync.dma_start(
                    rs_in[qq][tc_i * 128 : (tc_i + 1) * 128, esl], ys[:]
                )

            def rs_quarter(qq):
                nc.gpsimd.collective_compute(
                    "ReduceScatter",
                    ADD,
                    replica_groups=PAIRS,
                    ins=[rs_in[qq].opt()],
                    outs=[rs_out[qq].opt()],
                )

            def out_copy(qq):
                # SWDGE copy; scheduled long after RS(qq) completed so its
                # wait never blocks the Pool stream
                nc.gpsimd.dma_start(out[qq * Q8 : (qq + 1) * Q8, :], rs_out[qq][:])

            # ---- background schedule (due_step, fn); run at END of step s.
            # A heap so evict tails can be enqueued dynamically mid-stream ----
            import heapq

            bg = []
            bg_seq = [0]

            def bg_push(due, fn):
                heapq.heappush(bg, (due, bg_seq[0], fn))
                bg_seq[0] += 1
            for tb in (1, 2, 3):
                bg_push(4 * tb - 2, lambda tb=tb: qkv_chain(4, tb))
                bg_push(4 * tb - 1, lambda tb=tb: (qkv_chain(8, tb), v_transpose(0, tb)))
            for p in (1, 2, 3):
                bg_push(16 * p - 4, lambda p=p: qkv_chain(p, 0))
                for tb in range(4):
                    bg_push(16 * p + 4 * tb - 2, lambda p=p, tb=tb: qkv_chain(4 + p, tb))
                    bg_push(
                        16 * p + 4 * tb - 1,
                        lambda p=p, tb=tb: (qkv_chain(8 + p, tb), v_transpose(p, tb)),
                    )
            bg_push(40, lambda: nc.sync.dma_start(wfc[:], w_fc[:]))
            for tb in (1, 2, 3):
                for p in range(4):
                    bg_push(64 * tb + 16 * p - 4, lambda p=p, tb=tb: qkv_chain(p, tb))
            for qq in range(3):
                items = [(tc_i, e2) for tc_i in range(4) for e2 in range(2)]
                for i, (tc_i, e2) in enumerate(items):
                    due = 64 * qq + 78 + 3 * i
                    is_last = i == len(items) - 1
                    def fitem(qq=qq, tc_i=tc_i, e2=e2, is_last=is_last):
                        fc_chain(qq, tc_i, e2)
                        if is_last:
                            rs_quarter(qq)
                    bg_push(due, fitem)
                if qq < 2:
                    bg_push(64 * qq + 150, lambda qq=qq: out_copy(qq))

            def run_due_bg(s):
                while bg and bg[0][0] <= s:
                    heapq.heappop(bg)[2]()

            # ---- prelude: warm the PE pstate on a dummy chain while input
            # DMAs stream, then head-pair 0, first 512 tokens ----
            warm = qp.tile([128, 512], f32, tag="ps", name="warm")
            for i in range(14):
                nc.tensor.matmul(
                    warm[:], wqp[:, 0, 0:128], xT[:, 0, 0:512],
                    start=(i == 0), stop=(i == 13),
                )
            qkv_chain(0, 0)
            qkv_chain(4, 0)
            qkv_chain(8, 0)
            v_transpose(0, 0)

            # ---- attention pipeline: 256 steps (1 key-tile x 2 heads) ----
            pts = {}
            av_tiles = {}

            def emit_av(s2):
                b_i, kk = divmod(s2, 16)
                qq, j = divmod(b_i, 4)
                if kk == 0:
                    av_tiles[b_i] = (
                        avp.tile([128, 512], f32, tag="avh0", name="avh0"),
                        avp.tile([128, 512], f32, tag="avh1", name="avh1"),
                    )
                pt = pts.pop(s2)
                for u, av in zip((0, 1), av_tiles[b_i]):
                    h = 2 * j + u
                    usl = slice(u * 512, (u + 1) * 512)
                    nc.tensor.matmul(
                        av[0:65, :],
                        v[:, h, kk, 0:65],
                        pt[:, usl],
                        start=(kk == 0),
                        stop=(kk == 15),
                    )
                if kk == 15:
                    evict_block(b_i, *av_tiles.pop(b_i))

            def evict_block(b_i, a0, a1):
                qq, j = divmod(b_i, 4)
                qsl = slice(qq * Q4, (qq + 1) * Q4)
                s_now = 16 * b_i + 16
                # immediate: one PSUM read per av bank frees them for the
                # next block without waiting on the normalize tail
                comb = wp.tile([128, 1024], f32, tag="comb", name="comb")
                nc.vector.tensor_copy(comb[0:65, 0:512], a0[0:65, :])
                nc.vector.tensor_copy(comb[0:65, 512:1024], a1[0:65, :])

                # sums shift -> reciprocal -> broadcast -> normalize. Deferred
                # for j<3 (FC reads it a quarter-late); inline for j==3 so the
                # quarter's FC chains aren't stuck behind a cold cross-engine
                # latency chain.
                def tail1():
                    srs = wp.tile([128, 2048], f32, tag="srs", name="srs")
                    nc.sync.dma_start(srs[0:1, 0:1024], comb[64:65, 0:1024])
                    nc.vector.reciprocal_approx_fast(
                        srs[0:1, 1024:2048], srs[0:1, 0:1024]
                    )
                    R = wp.tile([128, 1024], f32, tag="R", name="R")
                    nc.gpsimd.partition_broadcast(R[:], srs[0:1, 1024:2048])

                    def tail2(R=R):
                        nc.gpsimd.tensor_tensor(
                            onT[0:64, j, qsl], comb[0:64, 0:512], R[0:64, 0:512],
                            op=MUL,
                        )
                        tmp = wp.tile([64, 512], bf16, tag="tmp", name="tmp")
                        nc.gpsimd.tensor_tensor(
                            tmp[:], comb[0:64, 512:1024], R[0:64, 512:1024], op=MUL
                        )
                        nc.sync.dma_start(onT[64:128, j, qsl], tmp[:])

                    if j == 3:
                        tail2()
                    else:
                        bg_push(s_now + 4, tail2)

                if j == 3:
                    tail1()
                else:
                    bg_push(s_now + 2, tail1)

            for s in range(256):
                b_i, kk = divmod(s, 16)
                qq, j = divmod(b_i, 4)
                qsl = slice(qq * Q4, (qq + 1) * Q4)
                ksl = slice(kk * 128, (kk + 1) * 128)
                st = stp.tile([128, 1024], f32, tag="st", name="st")
                nc.tensor.matmul(
                    st[:, 0:512], kt[0:64, j, ksl], qt[0:64, j, qsl],
                    start=True, stop=True,
                )
                nc.tensor.matmul(
                    st[:, 512:1024], kt[64:128, j, ksl], qt[64:128, j, qsl],
                    start=True, stop=True,
                )
                pt = wp.tile([128, 1024], bf16, tag="pt", bufs=3, name="pt")
                nc.scalar.activation(pt[:], st[:], Exp, scale=0.125)
                pts[s] = pt
                if s > 0:
                    emit_av(s - 1)
                run_due_bg(s)
            emit_av(255)

            # ---- tail: FC + RS for the last quarter ----
            run_due_bg(10**9)
            for tc_i in range(4):
                for e2 in range(2):
                    fc_chain(3, tc_i, e2)
            rs_quarter(3)
            out_copy(2)
            out_copy(3)

    nc.finalize()
    return nc


def _prep_inputs(X, W_qkv, b_qkv, W_fc, b_fc):
    """Host-side shard + permute + cast. Returns in_maps for 8 cores."""
    X = np.asarray(X, dtype=np.float32)
    W_qkv = np.asarray(W_qkv, dtype=np.float32)
    b_qkv = np.asarray(b_qkv, dtype=np.float32)
    W_fc = np.asarray(W_fc, dtype=np.float32)
    b_fc = np.asarray(b_fc, dtype=np.float32)

    pre_cols = np.concatenate([np.arange(ft * 128, (ft + 1) * 128) for ft in PRE_FTS])
    rest_cols = np.concatenate([np.arange(ft * 128, (ft + 1) * 128) for ft in REST_FTS])

    in_maps = []
    bfc_half = (0.5 * b_fc).astype(np.float32).reshape(1, E)
    for c in range(NCORES):
        b, g = divmod(c, 2)
        heads = np.arange(g * H8, (g + 1) * H8)
        # column order: all Q feats (head-major), then K, then V
        cols = np.concatenate(
            [
                np.concatenate([h * 3 * DK + off + np.arange(DK) for h in heads])
                for off in (0, DK, 2 * DK)
            ]
        )
        wq_sh = W_qkv[:, cols].astype(ml_dtypes.bfloat16)
        bq_sh = b_qkv[cols].astype(np.float32).reshape(12, 128).T.copy()
        wfc_sh = W_fc[g * FO : (g + 1) * FO, :].astype(ml_dtypes.bfloat16)

        def sbuf_layout(arr, width):
            # [(a p), f] -> [p, (a f)] so the device DMA is contiguous
            a = arr.shape[0] // 128
            return np.ascontiguousarray(
                arr.reshape(a, 128, width).transpose(1, 0, 2).reshape(128, a * width)
            )

        in_maps.append(
            {
                "x": np.ascontiguousarray(X[b].T).astype(ml_dtypes.bfloat16),
                "w_pre": sbuf_layout(wq_sh[:, pre_cols], 384),
                "w_rest": sbuf_layout(wq_sh[:, rest_cols], 1152),
                "b_qkv": np.ascontiguousarray(bq_sh),
                "w_fc": sbuf_layout(wfc_sh, E),
                "b_fc": bfc_half,
            }
        )
    return in_maps


def run_kernel(inputs, trace=False):
    if "nc" not in _CACHE:
        _CACHE["nc"] = build_nc()
    nc = _CACHE["nc"]
    in_maps = _prep_inputs(**inputs)
    res = bass_utils.run_bass_kernel_spmd(
        nc, in_maps, core_ids=list(range(NCORES)), trace=trace
    )
    Y = np.empty((B, L, E), dtype=np.float32)
    for c in range(NCORES):
        b, g = divmod(c, 2)
        o = np.asarray(res.results[c]["out"]).astype(np.float32)
        for qq in range(4):
            Y[b, qq * Q4 + g * Q8 : qq * Q4 + (g + 1) * Q8, :] = o[
                qq * Q8 : (qq + 1) * Q8
            ]
    return Y, res


def kernel(X, W_qkv, b_qkv, W_fc, b_fc):
    Y, _ = run_kernel(
        dict(X=X, W_qkv=W_qkv, b_qkv=b_qkv, W_fc=W_fc, b_fc=b_fc), trace=False
    )
    return Y



# revision 4
# speedup vs baseline: 1.1216x; 1.1216x over previous
"""Multi-head attention (B=4, L=2048, E=1024, H=16, DK=64) on 8 TRN2 cores.

Sharding: core c -> (batch b = c//2, head-group g = c%2 of 8 heads).

v2: ACT-paced design. The exp on [128,1024] per step (~1.11us) is the
hard floor (33.5M exps/core, 1 elem/lane/cycle); all PE work is scheduled
to fit underneath it. Per step (1 key-tile x 2 heads x 512 queries):
fused ST pass (row-split, 512 cyc) -> exp (ACT) -> col-tiled packed AV
pass (2 heads via tile_position (0,0)/(0,64), 512 cyc) + a 4-col-tile
sums pass every 2 steps (ones-vector matmuls at positions 0/32/64/96,
512 cyc). Block order interleaves head-pairs/quarters so QKV chains and
FC/ReduceScatter spread evenly. gpsimd carries ONLY the per-block
reciprocal broadcast, the pairwise ReduceScatters and final out copies;
normalize runs on DVE with partition-aligned operands, so a blocking RS
never stalls the PE/ACT pipeline.

Self-contained: hardcodes all shapes; requires only the concourse stack.
"""

import numpy as np
import ml_dtypes

try:
    import axon_prof

    axon_prof.install()
except Exception:
    pass

import concourse.mybir as mybir
import concourse.tile as tile
from concourse import bacc
from concourse import bass_utils

B, L, E = 4, 2048, 1024
H, DK = 16, 64
H8 = 8                      # heads per core
F = H8 * 3 * DK             # qkv features per core = 1536
FO = H8 * DK                # attn-out features per core = 512
NCORES = 8
Q4 = L // 4                 # 512 queries per quarter
Q8 = Q4 // 2                # 256 tokens scattered to each pair member

# ft-tile order: Q tiles 0..3 (head-pairs), K tiles 4..7, V tiles 8..11.
# Tiles {0,4,8} (head-pair 0) ship in w_pre so pair-0 chains start early.
PRE_FTS = (0, 4, 8)
REST_FTS = (1, 2, 3, 5, 6, 7, 9, 10, 11)
PRE_IDX = {ft: i for i, ft in enumerate(PRE_FTS)}
REST_IDX = {ft: i for i, ft in enumerate(REST_FTS)}

# block order: pairs 0,1 quarter-major; pairs 2,3 interleaved by quarter
# so quarter qq completes at block 9+2qq and FC/RS spread 32 steps apart.
BLOCKS = [(0, 0), (0, 1), (0, 2), (0, 3), (1, 0), (1, 1), (1, 2), (1, 3),
          (2, 0), (3, 0), (2, 1), (3, 1), (2, 2), (3, 2), (2, 3), (3, 3)]
# first step at which quarter qq's onT is complete (end of block 9+2qq)
QDONE_STEP = {qq: 16 * (9 + 2 * qq) + 16 for qq in range(4)}

f32 = mybir.dt.float32
bf16 = mybir.dt.bfloat16
Exp = mybir.ActivationFunctionType.Exp
MUL = mybir.AluOpType.mult
ADD = mybir.AluOpType.add

_CACHE = {}


def build_nc():
    nc = bacc.Bacc("TRN2", target_bir_lowering=False, debug=False, num_devices=NCORES)

    # weight tensors arrive host-prearranged in SBUF layout (partition-major)
    x = nc.dram_tensor("x", [E, L], bf16, kind="ExternalInput")
    w_pre = nc.dram_tensor("w_pre", [128, 8 * 384], bf16, kind="ExternalInput")
    w_rest = nc.dram_tensor("w_rest", [128, 8 * 1152], bf16, kind="ExternalInput")
    b_qkv = nc.dram_tensor("b_qkv", [128, 12], f32, kind="ExternalInput")
    w_fc = nc.dram_tensor("w_fc", [128, 4 * E], bf16, kind="ExternalInput")
    b_fc = nc.dram_tensor("b_fc", [1, E], f32, kind="ExternalInput")
    # 4 quarters x 256 scattered tokens each; host casts bf16 -> f32
    out = nc.dram_tensor("out", [L // 2, E], bf16, kind="ExternalOutput")

    PAIRS = [[0, 1], [2, 3], [4, 5], [6, 7]]

    with tile.TileContext(nc) as tc:
        with (
            tc.tile_pool(name="persist", bufs=1) as pp,
            tc.tile_pool(name="work", bufs=1) as wp,
            tc.tile_pool(name="ys", bufs=3) as yp_pool,
            tc.tile_pool(name="stp", bufs=2, space="PSUM") as stp,
            tc.tile_pool(name="avp", bufs=1, space="PSUM") as avp,
            tc.tile_pool(name="smp", bufs=1, space="PSUM") as smp,
            tc.tile_pool(name="qp", bufs=2, space="PSUM") as qp,
            tc.tile_pool(name="dram", bufs=1, space="DRAM") as dram,
        ):
            # ---- persistent SBUF ----
            xT = pp.tile([128, 8, L], bf16, tag="xT")          # X^T  4 MiB
            wqp = pp.tile([128, 8, 384], bf16, tag="wqp")
            wqr = pp.tile([128, 8, 1152], bf16, tag="wqr")
            bq = pp.tile([128, 12], f32, tag="bq")
            wfc = pp.tile([128, 4, E], bf16, tag="wfc")        # 1 MiB
            bias = pp.tile([128, E], f32, tag="bias")          # 0.5 MiB
            qt = pp.tile([128, 4, L], bf16, tag="qt")          # Q^T 2 MiB
            kt = pp.tile([128, 4, L], bf16, tag="kt")          # K^T 2 MiB
            vt = pp.tile([128, 4, L], bf16, tag="vt")          # V^T 2 MiB
            v = pp.tile([128, H8, 16, 80], bf16, tag="v")      # 2.5 MiB
            onT = pp.tile([128, 4, L], bf16, tag="onT")        # attn out^T 2 MiB
            ones = pp.tile([128, 1], bf16, tag="ones")

            rs_in = [
                dram.tile([Q4, E], bf16, name=f"rs_in{i}", tag=f"rs_in{i}")
                for i in range(4)
            ]
            rs_out = [
                dram.tile([Q8, E], bf16, name=f"rs_out{i}", tag=f"rs_out{i}")
                for i in range(4)
            ]

            # ---- input DMAs on the two HWDGE queues (sync/scalar); each DMA
            # sprays across the SDMA engines, so ordering is what matters:
            # w_pre + token-half 0 of all xT chunks first so pair-0 chains
            # can start early. ----
            nc.scalar.dma_start(bq[:], b_qkv[:])
            bfc_row = pp.tile([1, E], f32, tag="bfc_row")
            nc.scalar.dma_start(bfc_row[:], b_fc[:])
            rings = [nc.sync, nc.scalar]
            for i in range(3):
                rings[i % 2].dma_start(
                    wqp[:, :, i * 128 : (i + 1) * 128],
                    w_pre[:, :].rearrange("p (a f) -> p a f", a=8)[
                        :, :, i * 128 : (i + 1) * 128
                    ],
                )
            for half in range(2):
                tsl = slice(half * 1024, (half + 1) * 1024)
                for e in range(8):
                    rings[e % 2].dma_start(
                        xT[:, e, tsl], x[e * 128 : (e + 1) * 128, tsl]
                    )
            nc.sync.dma_start(wqr[:], w_rest[:])
            nc.gpsimd.partition_broadcast(bias[:], bfc_row[:])
            nc.vector.memset(ones[:], 1.0)

            def wq_sl(ft):
                if ft in PRE_IDX:
                    i = PRE_IDX[ft]
                    return lambda kc: wqp[:, kc, i * 128 : (i + 1) * 128]
                i = REST_IDX[ft]
                return lambda kc: wqr[:, kc, i * 128 : (i + 1) * 128]

            # ---- emitters ----
            def qkv_chain(ft, tb):
                ps = qp.tile([128, 512], f32, tag="ps", name="ps")
                wa = wq_sl(ft)
                tsl = slice(tb * 512, (tb + 1) * 512)
                for kc in range(8):
                    nc.tensor.matmul(
                        ps[:], wa(kc), xT[:, kc, tsl],
                        start=(kc == 0), stop=(kc == 7),
                    )
                if ft < 4:
                    dst = qt[:, ft, tsl]
                elif ft < 8:
                    dst = kt[:, ft - 4, tsl]
                else:
                    dst = vt[:, ft - 8, tsl]
                nc.vector.tensor_scalar_add(dst, ps[:], bq[:, ft : ft + 1])

            def v_transpose(p, tb):
                for h in (2 * p, 2 * p + 1):
                    nc.sync.dma_start_transpose(
                        v[:, h, tb * 4 : (tb + 1) * 4, 0:DK],
                        vt[(h % 2) * 64 : (h % 2) * 64 + 64, p, tb * 512 : (tb + 1) * 512],
                    )

            def fc_chain(qq, tc_i, e2):
                t0 = (qq * 4 + tc_i) * 128
                esl = slice(e2 * 512, (e2 + 1) * 512)
                yp = qp.tile([128, 512], f32, tag="ps", name="yp")
                for c in range(4):
                    nc.tensor.matmul(
                        yp[:], onT[:, c, t0 : t0 + 128], wfc[:, c, esl],
                        start=(c == 0), stop=(c == 3),
                    )
                ys = yp_pool.tile([128, 512], bf16, tag="ys", name="ys")
                nc.vector.tensor_tensor(ys[:], yp[:], bias[:, esl], op=ADD)
                nc.sync.dma_start(
                    rs_in[qq][tc_i * 128 : (tc_i + 1) * 128, esl], ys[:]
                )

            def rs_quarter(qq):
                nc.gpsimd.collective_compute(
                    "ReduceScatter",
                    ADD,
                    replica_groups=PAIRS,
                    ins=[rs_in[qq].opt()],
                    outs=[rs_out[qq].opt()],
                )

            def out_copy(qq):
                nc.gpsimd.dma_start(out[qq * Q8 : (qq + 1) * Q8, :], rs_out[qq][:])

            # ---- background schedule (due_step, fn); runs at END of step s ----
            import heapq

            bg = []
            bg_seq = [0]

            def bg_push(due, fn):
                heapq.heappush(bg, (due, bg_seq[0], fn))
                bg_seq[0] += 1

            # K/V/Q chains; first uses: K(p,tb) @ fk[p]+4tb, Q(j,qq) @ 16*block
            for tb in (1, 2, 3):
                bg_push(4 * tb - 4, lambda tb=tb: qkv_chain(4, tb))
                bg_push(4 * tb - 3, lambda tb=tb: (qkv_chain(8, tb), v_transpose(0, tb)))
            for qq in (1, 2, 3):
                bg_push(16 * qq - 5, lambda qq=qq: qkv_chain(0, qq))
            for tb in range(4):
                bg_push(20 + 8 * tb, lambda tb=tb: qkv_chain(5, tb))
                bg_push(24 + 8 * tb, lambda tb=tb: (qkv_chain(9, tb), v_transpose(1, tb)))
                bg_push(96 + 4 * tb, lambda tb=tb: qkv_chain(6, tb))
                bg_push(98 + 4 * tb, lambda tb=tb: (qkv_chain(10, tb), v_transpose(2, tb)))
                bg_push(112 + 4 * tb, lambda tb=tb: qkv_chain(7, tb))
                bg_push(114 + 4 * tb, lambda tb=tb: (qkv_chain(11, tb), v_transpose(3, tb)))
            qdue = {(1, 0): 54, (1, 1): 72, (1, 2): 88, (1, 3): 104,
                    (2, 0): 120, (2, 1): 152, (2, 2): 184, (2, 3): 216,
                    (3, 0): 130, (3, 1): 168, (3, 2): 200, (3, 3): 232}
            for (j, qq), due in qdue.items():
                bg_push(due, lambda j=j, qq=qq: qkv_chain(j, qq))
            bg_push(40, lambda: nc.sync.dma_start(wfc[:], w_fc[:]))
            for qq in range(4):
                items = [(tc_i, e2) for tc_i in range(4) for e2 in range(2)]
                for i, (tc_i, e2) in enumerate(items):
                    bg_push(
                        QDONE_STEP[qq] + 4 + i,
                        lambda qq=qq, tc_i=tc_i, e2=e2: fc_chain(qq, tc_i, e2),
                    )
                bg_push(QDONE_STEP[qq] + 13, lambda qq=qq: rs_quarter(qq))
                bg_push(QDONE_STEP[qq] + 33, lambda qq=qq: out_copy(qq))

            def run_due_bg(s):
                while bg and bg[0][0] <= s:
                    heapq.heappop(bg)[2]()

            # ---- prelude: warm the PE on a dummy chain while inputs stream ----
            warm = qp.tile([128, 512], f32, tag="ps", name="warm")
            for i in range(14):
                nc.tensor.matmul(
                    warm[:], wqp[:, 0, 0:128], xT[:, 0, 0:512],
                    start=(i == 0), stop=(i == 13),
                )
            qkv_chain(4, 0)   # K(pair0, tb0)
            qkv_chain(0, 0)   # Q(pair0, quarter0)
            qkv_chain(8, 0)   # V(pair0, tb0)
            v_transpose(0, 0)

            # ---- attention pipeline: 256 steps ----
            pts = {}
            av_tiles = {}
            sm_tiles = {}
            # sums row position per (head-half u, kk parity)
            SUMROW = {(0, 0): 0, (0, 1): 32, (1, 0): 64, (1, 1): 96}

            def emit_av(s2):
                b_i, kk = divmod(s2, 16)
                j, qq = BLOCKS[b_i]
                if kk == 0:
                    av_tiles[b_i] = avp.tile([128, 512], f32, tag="av", name="av")
                    sm_tiles[b_i] = smp.tile([128, 512], f32, tag="sm", name="sm")
                av = av_tiles[b_i]
                sm = sm_tiles[b_i]
                pt = pts[s2]
                for u in (0, 1):
                    h = 2 * j + u
                    usl = slice(u * 512, (u + 1) * 512)
                    nc.tensor.matmul(
                        av[u * 64 : (u + 1) * 64, :],
                        v[:, h, kk, 0:DK],
                        pt[:, usl],
                        start=(kk == 0),
                        stop=(kk == 15),
                        tile_position=(0, u * 64),
                    )
                if kk % 2 == 1:
                    pt_prev = pts.pop(s2 - 1)
                    for u in (0, 1):
                        usl = slice(u * 512, (u + 1) * 512)
                        for par, ptx in ((0, pt_prev), (1, pt)):
                            row = SUMROW[(u, par)]
                            nc.tensor.matmul(
                                sm[row : row + 1, :],
                                ones[:],
                                ptx[:, usl],
                                start=(kk == 1),
                                stop=(kk == 15),
                                tile_position=(0, row),
                            )
                if kk == 15:
                    pts.pop(s2)
                    evict_block(b_i, av_tiles.pop(b_i), sm_tiles.pop(b_i))

            def evict_block(b_i, av, sm):
                j, qq = BLOCKS[b_i]
                qsl = slice(qq * Q4, (qq + 1) * Q4)
                s_now = 16 * b_i + 16
                # one PSUM read per bank frees them for the next block
                comb = wp.tile([128, 512], f32, tag="comb", name="comb")
                nc.vector.tensor_copy(comb[:], av[:])
                sumsb = wp.tile([128, 512], f32, tag="sumsb", name="sumsb")
                nc.vector.tensor_copy(sumsb[0:97, :], sm[0:97, :])
                # gather the 4 sums rows into one partition: (h0e|h1e|h0o|h1o)
                srs = wp.tile([1, 2048], f32, tag="srs", name="srs")
                nc.sync.dma_start(srs[0:1, 0:512], sumsb[0:1, :])
                nc.sync.dma_start(srs[0:1, 512:1024], sumsb[64:65, :])
                nc.sync.dma_start(srs[0:1, 1024:1536], sumsb[32:33, :])
                nc.sync.dma_start(srs[0:1, 1536:2048], sumsb[96:97, :])
                s2t = wp.tile([1, 1024], f32, tag="s2t", name="s2t")
                nc.vector.tensor_tensor(
                    s2t[:], srs[0:1, 0:1024], srs[0:1, 1024:2048], op=ADD
                )
                r2 = wp.tile([1, 1024], f32, tag="r2", name="r2")
                nc.vector.reciprocal_approx_fast(r2[:], s2t[:])
                R = wp.tile([128, 1024], f32, tag="R", name="R")
                nc.gpsimd.partition_broadcast(R[:], r2[:])

                def norm(j=j, qsl=qsl, comb=comb, R=R):
                    nc.vector.tensor_tensor(
                        onT[0:64, j, qsl], comb[0:64, :], R[0:64, 0:512], op=MUL
                    )
                    nc.vector.tensor_tensor(
                        onT[64:128, j, qsl], comb[64:128, :], R[64:128, 512:1024],
                        op=MUL,
                    )

                bg_push(s_now + 2, norm)

            for s in range(256):
                b_i, kk = divmod(s, 16)
                j, qq = BLOCKS[b_i]
                qsl = slice(qq * Q4, (qq + 1) * Q4)
                ksl = slice(kk * 128, (kk + 1) * 128)
                st = stp.tile([128, 1024], f32, tag="st", name="st")
                nc.tensor.matmul(
                    st[:, 0:512], kt[0:64, j, ksl], qt[0:64, j, qsl],
                    start=True, stop=True,
                )
                nc.tensor.matmul(
                    st[:, 512:1024], kt[64:128, j, ksl], qt[64:128, j, qsl],
                    start=True, stop=True,
                )
                pt = wp.tile([128, 1024], bf16, tag="pt", bufs=4, name="pt")
                nc.scalar.activation(pt[:], st[:], Exp, scale=0.125)
                pts[s] = pt
                if s > 0:
                    emit_av(s - 1)
                run_due_bg(s)
            emit_av(255)

            # ---- tail: norm(15), FC + RS for quarter 3, out copies ----
            run_due_bg(10**9)

    nc.finalize()
    return nc


def _prep_inputs(X, W_qkv, b_qkv, W_fc, b_fc):
    """Host-side shard + permute + cast. Returns in_maps for 8 cores."""
    X = np.asarray(X, dtype=np.float32)
    W_qkv = np.asarray(W_qkv, dtype=np.float32)
    b_qkv = np.asarray(b_qkv, dtype=np.float32)
    W_fc = np.asarray(W_fc, dtype=np.float32)
    b_fc = np.asarray(b_fc, dtype=np.float32)

    pre_cols = np.concatenate([np.arange(ft * 128, (ft + 1) * 128) for ft in PRE_FTS])
    rest_cols = np.concatenate([np.arange(ft * 128, (ft + 1) * 128) for ft in REST_FTS])

    in_maps = []
    bfc_half = (0.5 * b_fc).astype(np.float32).reshape(1, E)
    for c in range(NCORES):
        b, g = divmod(c, 2)
        heads = np.arange(g * H8, (g + 1) * H8)
        # column order: all Q feats (head-major), then K, then V
        cols = np.concatenate(
            [
                np.concatenate([h * 3 * DK + off + np.arange(DK) for h in heads])
                for off in (0, DK, 2 * DK)
            ]
        )
        wq_sh = W_qkv[:, cols].astype(ml_dtypes.bfloat16)
        bq_sh = b_qkv[cols].astype(np.float32).reshape(12, 128).T.copy()
        wfc_sh = W_fc[g * FO : (g + 1) * FO, :].astype(ml_dtypes.bfloat16)

        def sbuf_layout(arr, width):
            # [(a p), f] -> [p, (a f)] so the device DMA is contiguous
            a = arr.shape[0] // 128
            return np.ascontiguousarray(
                arr.reshape(a, 128, width).transpose(1, 0, 2).reshape(128, a * width)
            )

        in_maps.append(
            {
                "x": np.ascontiguousarray(X[b].T).astype(ml_dtypes.bfloat16),
                "w_pre": sbuf_layout(wq_sh[:, pre_cols], 384),
                "w_rest": sbuf_layout(wq_sh[:, rest_cols], 1152),
                "b_qkv": np.ascontiguousarray(bq_sh),
                "w_fc": sbuf_layout(wfc_sh, E),
                "b_fc": bfc_half,
            }
        )
    return in_maps


def run_kernel(inputs, trace=False):
    if "nc" not in _CACHE:
        _CACHE["nc"] = build_nc()
    nc = _CACHE["nc"]
    in_maps = _prep_inputs(**inputs)
    res = bass_utils.run_bass_kernel_spmd(
        nc, in_maps, core_ids=list(range(NCORES)), trace=trace
    )
    Y = np.empty((B, L, E), dtype=np.float32)
    for c in range(NCORES):
        b, g = divmod(c, 2)
        o = np.asarray(res.results[c]["out"]).astype(np.float32)
        for qq in range(4):
            Y[b, qq * Q4 + g * Q8 : qq * Q4 + (g + 1) * Q8, :] = o[
                qq * Q8 : (qq + 1) * Q8
            ]
    return Y, res


def kernel(X, W_qkv, b_qkv, W_fc, b_fc):
    Y, _ = run_kernel(
        dict(X=X, W_qkv=W_qkv, b_qkv=b_qkv, W_fc=W_fc, b_fc=b_fc), trace=False
    )
    return Y


# revision 10
# speedup vs baseline: 1.2125x; 1.0810x over previous
"""Multi-head attention (B=4, L=2048, E=1024, H=16, DK=64) on 8 TRN2 cores.

Sharding: core c -> (batch b = c//2, head-group g = c%2 of 8 heads).

v2: ACT-paced design. The exp on [128,1024] per step (~1.11us) is the
hard floor (33.5M exps/core, 1 elem/lane/cycle); all PE work is scheduled
to fit underneath it. Per step (1 key-tile x 2 heads x 512 queries):
fused ST pass (row-split, 512 cyc) -> exp (ACT) -> col-tiled packed AV
pass (2 heads via tile_position (0,0)/(0,64), 512 cyc) + a 4-col-tile
sums pass every 2 steps (ones-vector matmuls at positions 0/32/64/96,
512 cyc). Block order interleaves head-pairs/quarters so QKV chains and
FC/ReduceScatter spread evenly. gpsimd carries ONLY the per-block
reciprocal broadcast, the pairwise ReduceScatters and final out copies;
normalize runs on DVE with partition-aligned operands, so a blocking RS
never stalls the PE/ACT pipeline.

Self-contained: hardcodes all shapes; requires only the concourse stack.
"""

import numpy as np
import ml_dtypes

try:
    import axon_prof

    axon_prof.install()
except Exception:
    pass

import concourse.mybir as mybir
import concourse.tile as tile
from concourse import bacc
from concourse import bass_utils

B, L, E = 4, 2048, 1024
H, DK = 16, 64
H8 = 8                      # heads per core
F = H8 * 3 * DK             # qkv features per core = 1536
FO = H8 * DK                # attn-out features per core = 512
NCORES = 8
Q4 = L // 4                 # 512 queries per quarter
Q8 = Q4 // 2                # 256 tokens scattered to each pair member

# ft-tile order: Q tiles 0..3 (head-pairs), K tiles 4..7, V tiles 8..11.
# Tiles {0,4,8} (head-pair 0) ship in w_pre so pair-0 chains start early.
PRE_FTS = (0, 4, 8)
REST_FTS = (1, 2, 3, 5, 6, 7, 9, 10, 11)
PRE_IDX = {ft: i for i, ft in enumerate(PRE_FTS)}
REST_IDX = {ft: i for i, ft in enumerate(REST_FTS)}

# block order: pairs 0,1 quarter-major; pairs 2,3 interleaved by quarter
# so quarter qq completes at block 9+2qq and FC/RS spread 32 steps apart.
BLOCKS = [(0, 0), (0, 1), (0, 2), (0, 3), (1, 0), (1, 1), (1, 2), (1, 3),
          (2, 0), (3, 0), (2, 1), (3, 1), (2, 2), (3, 2), (2, 3), (3, 3)]
# first step at which quarter qq's onT is complete (end of block 9+2qq)
QDONE_STEP = {qq: 16 * (9 + 2 * qq) + 16 for qq in range(4)}

f32 = mybir.dt.float32
bf16 = mybir.dt.bfloat16
Exp = mybir.ActivationFunctionType.Exp
MUL = mybir.AluOpType.mult
ADD = mybir.AluOpType.add

_CACHE = {}


def build_nc():
    nc = bacc.Bacc("TRN2", target_bir_lowering=False, debug=False, num_devices=NCORES)

    # weight tensors arrive host-prearranged in SBUF layout (partition-major)
    x = nc.dram_tensor("x", [E, L], bf16, kind="ExternalInput")
    w_pre = nc.dram_tensor("w_pre", [128, 8 * 384], bf16, kind="ExternalInput")
    w_rest = nc.dram_tensor("w_rest", [128, 8 * 1152], bf16, kind="ExternalInput")
    b_qkv = nc.dram_tensor("b_qkv", [128, 12], f32, kind="ExternalInput")
    w_fc = nc.dram_tensor("w_fc", [128, 4 * E], bf16, kind="ExternalInput")
    b_fc = nc.dram_tensor("b_fc", [1, E], f32, kind="ExternalInput")
    # 4 quarters x 256 scattered tokens each; host casts bf16 -> f32
    out = nc.dram_tensor("out", [L // 2, E], bf16, kind="ExternalOutput")

    PAIRS = [[0, 1], [2, 3], [4, 5], [6, 7]]

    with tile.TileContext(nc) as tc:
        with (
            tc.tile_pool(name="persist", bufs=1) as pp,
            tc.tile_pool(name="work", bufs=1) as wp,
            tc.tile_pool(name="ys", bufs=3) as yp_pool,
            tc.tile_pool(name="stp", bufs=2, space="PSUM") as stp,
            tc.tile_pool(name="avp", bufs=1, space="PSUM") as avp,
            tc.tile_pool(name="smp", bufs=1, space="PSUM") as smp,
            tc.tile_pool(name="qp", bufs=2, space="PSUM") as qp,
            tc.tile_pool(name="dram", bufs=1, space="DRAM") as dram,
        ):
            # ---- persistent SBUF ----
            xT = pp.tile([128, 8, L], bf16, tag="xT")          # X^T  4 MiB
            wqp = pp.tile([128, 8, 384], bf16, tag="wqp")
            wqr = pp.tile([128, 8, 1152], bf16, tag="wqr")
            bq = pp.tile([128, 12], f32, tag="bq")
            wfc = pp.tile([128, 4, E], bf16, tag="wfc")        # 1 MiB
            bias = pp.tile([128, E], f32, tag="bias")          # 0.5 MiB
            qt = pp.tile([128, 4, L], bf16, tag="qt")          # Q^T 2 MiB
            kt = pp.tile([128, 4, L], bf16, tag="kt")          # K^T 2 MiB
            vt = pp.tile([128, 4, L], bf16, tag="vt")          # V^T 2 MiB
            v = pp.tile([128, H8, 16, 80], bf16, tag="v")      # 2.5 MiB
            onT = pp.tile([128, 4, L], bf16, tag="onT")        # attn out^T 2 MiB
            ones = pp.tile([128, 1], bf16, tag="ones")

            rs_in = [
                dram.tile([Q4, E], bf16, name=f"rs_in{i}", tag=f"rs_in{i}")
                for i in range(4)
            ]
            rs_out = [
                dram.tile([Q8, E], bf16, name=f"rs_out{i}", tag=f"rs_out{i}")
                for i in range(4)
            ]

            # ---- input DMAs on the two HWDGE queues (sync/scalar); each DMA
            # sprays across the SDMA engines, so ordering is what matters:
            # w_pre + token-half 0 of all xT chunks first so pair-0 chains
            # can start early. ----
            nc.scalar.dma_start(bq[:], b_qkv[:])
            bfc_row = pp.tile([1, E], f32, tag="bfc_row")
            nc.scalar.dma_start(bfc_row[:], b_fc[:])
            rings = [nc.sync, nc.scalar]
            for i in range(3):
                rings[i % 2].dma_start(
                    wqp[:, :, i * 128 : (i + 1) * 128],
                    w_pre[:, :].rearrange("p (a f) -> p a f", a=8)[
                        :, :, i * 128 : (i + 1) * 128
                    ],
                )
            for half in range(2):
                tsl = slice(half * 1024, (half + 1) * 1024)
                for e in range(8):
                    rings[e % 2].dma_start(
                        xT[:, e, tsl], x[e * 128 : (e + 1) * 128, tsl]
                    )
            nc.sync.dma_start(wqr[:], w_rest[:])
            nc.gpsimd.partition_broadcast(bias[:], bfc_row[:])
            nc.vector.memset(ones[:], 1.0)

            def wq_sl(ft):
                if ft in PRE_IDX:
                    i = PRE_IDX[ft]
                    return lambda kc: wqp[:, kc, i * 128 : (i + 1) * 128]
                i = REST_IDX[ft]
                return lambda kc: wqr[:, kc, i * 128 : (i + 1) * 128]

            # ---- emitters ----
            def qkv_chain(ft, tb):
                ps = qp.tile([128, 512], f32, tag="ps", name="ps")
                wa = wq_sl(ft)
                tsl = slice(tb * 512, (tb + 1) * 512)
                for kc in range(8):
                    nc.tensor.matmul(
                        ps[:], wa(kc), xT[:, kc, tsl],
                        start=(kc == 0), stop=(kc == 7),
                    )
                if ft < 4:
                    dst = qt[:, ft, tsl]
                elif ft < 8:
                    dst = kt[:, ft - 4, tsl]
                else:
                    dst = vt[:, ft - 8, tsl]
                nc.vector.tensor_scalar_add(dst, ps[:], bq[:, ft : ft + 1])

            def v_transpose(p, tb):
                for h in (2 * p, 2 * p + 1):
                    nc.sync.dma_start_transpose(
                        v[:, h, tb * 4 : (tb + 1) * 4, 0:DK],
                        vt[(h % 2) * 64 : (h % 2) * 64 + 64, p, tb * 512 : (tb + 1) * 512],
                    )

            def fc_chain(qq, tc_i, e2):
                t0 = (qq * 4 + tc_i) * 128
                esl = slice(e2 * 512, (e2 + 1) * 512)
                yp = qp.tile([128, 512], f32, tag="ps", name="yp")
                for c in range(4):
                    nc.tensor.matmul(
                        yp[:], onT[:, c, t0 : t0 + 128], wfc[:, c, esl],
                        start=(c == 0), stop=(c == 3),
                    )
                ys = yp_pool.tile([128, 512], bf16, tag="ys", name="ys")
                nc.vector.tensor_tensor(ys[:], yp[:], bias[:, esl], op=ADD)
                nc.sync.dma_start(
                    rs_in[qq][tc_i * 128 : (tc_i + 1) * 128, esl], ys[:]
                )

            def rs_quarter(qq):
                nc.gpsimd.collective_compute(
                    "ReduceScatter",
                    ADD,
                    replica_groups=PAIRS,
                    ins=[rs_in[qq].opt()],
                    outs=[rs_out[qq].opt()],
                )

            def out_copy(qq):
                nc.gpsimd.dma_start(out[qq * Q8 : (qq + 1) * Q8, :], rs_out[qq][:])

            # ---- background schedule (due_step, fn); runs at END of step s ----
            import heapq

            bg = []
            bg_seq = [0]

            def bg_push(due, fn):
                heapq.heappush(bg, (due, bg_seq[0], fn))
                bg_seq[0] += 1

            # K/V/Q chains; first uses: K(p,tb) @ fk[p]+4tb, Q(j,qq) @ 16*block
            kdue = {(0, 1): 0, (0, 2): 2, (0, 3): 6,
                    (1, 0): 14, (1, 1): 24, (1, 2): 34, (1, 3): 44,
                    (2, 0): 88, (2, 1): 96, (2, 2): 104, (2, 3): 112,
                    (3, 0): 120, (3, 1): 128, (3, 2): 134, (3, 3): 140}
            vdue = {(0, 0): 1, (0, 1): 4, (0, 2): 8, (0, 3): 12,
                    (1, 0): 18, (1, 1): 28, (1, 2): 38, (1, 3): 48,
                    (2, 0): 92, (2, 1): 100, (2, 2): 108, (2, 3): 116,
                    (3, 0): 124, (3, 1): 131, (3, 2): 137, (3, 3): 143}
            for (p, tb), due in kdue.items():
                bg_push(due, lambda p=p, tb=tb: qkv_chain(4 + p, tb))
            for (p, tb), due in vdue.items():
                bg_push(due, lambda p=p, tb=tb: (qkv_chain(8 + p, tb), v_transpose(p, tb)))
            qdue = {(0, 1): 10, (0, 2): 16, (0, 3): 32,
                    (1, 0): 52, (1, 1): 68, (1, 2): 84, (1, 3): 100,
                    (2, 0): 118, (2, 1): 150, (2, 2): 182, (2, 3): 214,
                    (3, 0): 136, (3, 1): 166, (3, 2): 198, (3, 3): 230}
            for (j, qq), due in qdue.items():
                bg_push(due, lambda j=j, qq=qq: qkv_chain(j, qq))
            bg_push(40, lambda: nc.sync.dma_start(wfc[:], w_fc[:]))
            # RS(qq) dues sit just after the R-broadcast of the block that
            # would otherwise queue behind the RS on gpsimd.
            rs_due = {0: 177, 1: 209, 2: 241, 3: 270}
            for qq in range(4):
                items = [(tc_i, e2) for tc_i in range(4) for e2 in range(2)]
                for i, (tc_i, e2) in enumerate(items):
                    bg_push(
                        QDONE_STEP[qq] + 4 + i,
                        lambda qq=qq, tc_i=tc_i, e2=e2: fc_chain(qq, tc_i, e2),
                    )
                bg_push(rs_due[qq], lambda qq=qq: rs_quarter(qq))
                bg_push(rs_due[qq] + 20, lambda qq=qq: out_copy(qq))

            def run_due_bg(s):
                while bg and bg[0][0] <= s:
                    heapq.heappop(bg)[2]()

            # ---- prelude: dep-free dummy chain warms the PE clock (HAM) at
            # t=0 while inputs stream; then only K/Q of (pair0, quarter0)
            # gate the first exp. ----
            dummy = pp.tile([128, 512], bf16, tag="dummy")
            nc.vector.memset(dummy[:], 0.25)
            warm = qp.tile([128, 512], f32, tag="ps", name="warm")
            for i in range(16):
                nc.tensor.matmul(
                    warm[:], dummy[:, 0:128], dummy[:],
                    start=(i == 0), stop=(i == 15),
                )
            qkv_chain(4, 0)   # K(pair0, tb0)
            qkv_chain(0, 0)   # Q(pair0, quarter0)

            # ---- attention pipeline: 256 steps ----
            pts = {}
            av_tiles = {}
            sm_tiles = {}
            # sums row position per (head-half u, kk parity)
            SUMROW = {(0, 0): 0, (0, 1): 32, (1, 0): 64, (1, 1): 96}

            def emit_av_quad(sbase):
                # AV for steps sbase..sbase+3 as 4 same-config col-packed
                # groups back-to-back, then the 2 sums groups; batching
                # same-shape groups lets consecutive passes pipeline.
                b_i, kk0 = divmod(sbase, 16)
                j, qq = BLOCKS[b_i]
                if kk0 == 0:
                    av_tiles[b_i] = avp.tile([128, 512], f32, tag="av", name="av")
                    sm_tiles[b_i] = smp.tile([128, 512], f32, tag="sm", name="sm")
                av = av_tiles[b_i]
                sm = sm_tiles[b_i]
                for kk in range(kk0, kk0 + 4):
                    pt = pts[16 * b_i + kk]
                    for u in (0, 1):
                        h = 2 * j + u
                        usl = slice(u * 512, (u + 1) * 512)
                        nc.tensor.matmul(
                            av[u * 64 : (u + 1) * 64, :],
                            v[:, h, kk, 0:DK],
                            pt[:, usl],
                            start=(kk == 0),
                            stop=(kk == 15),
                            tile_position=(0, u * 64),
                        )
                for kke in (kk0, kk0 + 2):
                    pt_e = pts.pop(16 * b_i + kke)
                    pt_o = pts.pop(16 * b_i + kke + 1)
                    for u in (0, 1):
                        usl = slice(u * 512, (u + 1) * 512)
                        for par, ptx in ((0, pt_e), (1, pt_o)):
                            row = SUMROW[(u, par)]
                            nc.tensor.matmul(
                                sm[row : row + 1, :],
                                ones[:],
                                ptx[:, usl],
                                start=(kke == 0),
                                stop=(kke == 14),
                                tile_position=(0, row),
                            )
                if kk0 == 12:
                    evict_block(b_i, av_tiles.pop(b_i), sm_tiles.pop(b_i))

            def evict_block(b_i, av, sm):
                j, qq = BLOCKS[b_i]
                qsl = slice(qq * Q4, (qq + 1) * Q4)
                s_now = 16 * b_i + 16
                # one PSUM read per bank frees them for the next block
                comb = wp.tile([128, 512], f32, tag="comb", name="comb")
                nc.vector.tensor_copy(comb[:], av[:])
                sumsb = wp.tile([128, 512], f32, tag="sumsb", name="sumsb")
                nc.vector.tensor_copy(sumsb[0:97, :], sm[0:97, :])
                # gather the 4 sums rows into one partition: (h0e|h1e|h0o|h1o)
                srs = wp.tile([1, 2048], f32, tag="srs", name="srs")
                nc.sync.dma_start(srs[0:1, 0:512], sumsb[0:1, :])
                nc.sync.dma_start(srs[0:1, 512:1024], sumsb[64:65, :])
                nc.sync.dma_start(srs[0:1, 1024:1536], sumsb[32:33, :])
                nc.sync.dma_start(srs[0:1, 1536:2048], sumsb[96:97, :])
                s2t = wp.tile([1, 1024], f32, tag="s2t", name="s2t")
                nc.vector.tensor_tensor(
                    s2t[:], srs[0:1, 0:1024], srs[0:1, 1024:2048], op=ADD
                )
                r2 = wp.tile([1, 1024], f32, tag="r2", name="r2")
                nc.vector.reciprocal_approx_fast(r2[:], s2t[:])
                R = wp.tile([128, 1024], f32, tag="R", name="R")
                nc.gpsimd.partition_broadcast(R[:], r2[:])

                def norm(j=j, qsl=qsl, comb=comb, R=R):
                    nc.vector.tensor_tensor(
                        onT[0:64, j, qsl], comb[0:64, :], R[0:64, 0:512], op=MUL
                    )
                    nc.vector.tensor_tensor(
                        onT[64:128, j, qsl], comb[64:128, :], R[64:128, 512:1024],
                        op=MUL,
                    )

                bg_push(s_now + 2, norm)

            for t in range(128):
                s0 = 2 * t
                # ST pair for both steps first (same config -> pipelined),
                # then both exps on ACT.
                sts = []
                for s in (s0, s0 + 1):
                    b_i, kk = divmod(s, 16)
                    j, qq = BLOCKS[b_i]
                    qsl = slice(qq * Q4, (qq + 1) * Q4)
                    ksl = slice(kk * 128, (kk + 1) * 128)
                    st = stp.tile([128, 1024], f32, tag="st", name="st")
                    nc.tensor.matmul(
                        st[:, 0:512], kt[0:64, j, ksl], qt[0:64, j, qsl],
                        start=True, stop=True,
                    )
                    nc.tensor.matmul(
                        st[:, 512:1024], kt[64:128, j, ksl], qt[64:128, j, qsl],
                        start=True, stop=True,
                    )
                    sts.append(st)
                for s, st in zip((s0, s0 + 1), sts):
                    pt = wp.tile([128, 1024], bf16, tag="pt", bufs=6, name="pt")
                    nc.scalar.activation(pt[:], st[:], Exp, scale=0.125)
                    pts[s] = pt
                if s0 % 4 == 0 and s0 >= 4:
                    emit_av_quad(s0 - 4)
                run_due_bg(s0 + 1)
            emit_av_quad(252)

            # ---- tail: norm(15), FC + RS for quarter 3, out copies ----
            run_due_bg(10**9)

    nc.finalize()
    return nc


def _prep_inputs(X, W_qkv, b_qkv, W_fc, b_fc):
    """Host-side shard + permute + cast. Returns in_maps for 8 cores."""
    X = np.asarray(X, dtype=np.float32)
    W_qkv = np.asarray(W_qkv, dtype=np.float32)
    b_qkv = np.asarray(b_qkv, dtype=np.float32)
    W_fc = np.asarray(W_fc, dtype=np.float32)
    b_fc = np.asarray(b_fc, dtype=np.float32)

    pre_cols = np.concatenate([np.arange(ft * 128, (ft + 1) * 128) for ft in PRE_FTS])
    rest_cols = np.concatenate([np.arange(ft * 128, (ft + 1) * 128) for ft in REST_FTS])

    in_maps = []
    bfc_half = (0.5 * b_fc).astype(np.float32).reshape(1, E)
    for c in range(NCORES):
        b, g = divmod(c, 2)
        heads = np.arange(g * H8, (g + 1) * H8)
        # column order: all Q feats (head-major), then K, then V
        cols = np.concatenate(
            [
                np.concatenate([h * 3 * DK + off + np.arange(DK) for h in heads])
                for off in (0, DK, 2 * DK)
            ]
        )
        wq_sh = W_qkv[:, cols].astype(ml_dtypes.bfloat16)
        bq_sh = b_qkv[cols].astype(np.float32).reshape(12, 128).T.copy()
        wfc_sh = W_fc[g * FO : (g + 1) * FO, :].astype(ml_dtypes.bfloat16)

        def sbuf_layout(arr, width):
            # [(a p), f] -> [p, (a f)] so the device DMA is contiguous
            a = arr.shape[0] // 128
            return np.ascontiguousarray(
                arr.reshape(a, 128, width).transpose(1, 0, 2).reshape(128, a * width)
            )

        in_maps.append(
            {
                "x": np.ascontiguousarray(X[b].T).astype(ml_dtypes.bfloat16),
                "w_pre": sbuf_layout(wq_sh[:, pre_cols], 384),
                "w_rest": sbuf_layout(wq_sh[:, rest_cols], 1152),
                "b_qkv": np.ascontiguousarray(bq_sh),
                "w_fc": sbuf_layout(wfc_sh, E),
                "b_fc": bfc_half,
            }
        )
    return in_maps


def run_kernel(inputs, trace=False):
    if "nc" not in _CACHE:
        _CACHE["nc"] = build_nc()
    nc = _CACHE["nc"]
    in_maps = _prep_inputs(**inputs)
    res = bass_utils.run_bass_kernel_spmd(
        nc, in_maps, core_ids=list(range(NCORES)), trace=trace
    )
    Y = np.empty((B, L, E), dtype=np.float32)
    for c in range(NCORES):
        b, g = divmod(c, 2)
        o = np.asarray(res.results[c]["out"]).astype(np.float32)
        for qq in range(4):
            Y[b, qq * Q4 + g * Q8 : qq * Q4 + (g + 1) * Q8, :] = o[
                qq * Q8 : (qq + 1) * Q8
            ]
    return Y, res


def kernel(X, W_qkv, b_qkv, W_fc, b_fc):
    Y, _ = run_kernel(
        dict(X=X, W_qkv=W_qkv, b_qkv=b_qkv, W_fc=W_fc, b_fc=b_fc), trace=False
    )
    return Y
